# revision 5
# baseline (speedup 1.0000x reference)
"""DescrptSeA descriptor kernel for 8 Trainium2 NeuronCores.

Data-parallel over the nloc axis (512 atoms/core). The neighbor gather runs
on-device via a flat jnp.take (the take_along_axis form trips a neuron
compiler assert; the flat form compiles). Wire traffic is minimized: in go
int16 neighbor indices with the mask folded into the sign bit (~2.3 MB),
replicated coords (~1.6 MB) and tiny weight tables; out comes only the
rank-4 factor xyz = rr^T @ gg per atom ([nf, 512, 4, 100] bf16, ~6.5 MB)
instead of the full 26-52 MB descriptor. The final res = xyz^T @ xyz[:, :16]
outer product is cheap (52 MFLOP) and runs on host BLAS in fp32.
"""

import numpy as np
import jax
import jax.numpy as jnp

NF, NLOC, NALL = 2, 4096, 8192
NTYPES = 2
SEL = [46, 92]
NNEI = sum(SEL)
SEC = [0, 46, 138]
NEURON = [25, 50, 100]
AXIS = 16
RCUT, RCUT_SMTH = 6.0, 0.5
PROT = 1e-6

NCORES = 8
SHARD = NLOC // NCORES  # 512 atoms per core


def _smooth_weight(d, rmin, rmax):
    uu = (d - rmin) / (rmax - rmin)
    uu = jnp.clip(uu, 0.0, 1.0)
    return uu * uu * uu * (-6.0 * uu * uu + 15.0 * uu - 10.0) + 1.0


def _shard_fn(nl_i16, coord_all, centers, atype_loc, mean, std,
              w0, b0, w1, b1, w2, b2):
    # nl_i16 [nf, shard, nnei] int16: frame-offset index, negative = padded
    # coord_all [nf*nall, 3] f32; centers [nf, shard, 3]
    nf, nloc, nnei = nl_i16.shape
    mask = (nl_i16 >= 0)
    nl = jnp.where(mask, nl_i16, 0).astype(jnp.int32)
    m = mask[..., None].astype(jnp.float32)

    coord_r = jnp.take(coord_all, nl.reshape(-1), axis=0)
    coord_r = coord_r.reshape(nf, nloc, nnei, 3)
    diff = coord_r - centers[:, :, None, :]
    length = jnp.sqrt(jnp.sum(diff * diff, axis=-1, keepdims=True))
    length = length * m + (1.0 - m)
    t0 = 1.0 / (length + PROT)
    t1 = diff / ((length + PROT) ** 2)
    w = _smooth_weight(length, RCUT_SMTH, RCUT) * m
    env = jnp.concatenate([t0, t1], axis=-1) * w  # [nf, shard, nnei, 4]

    is0 = (atype_loc == 0)[:, :, None, None]
    mean_sel = jnp.where(is0, mean[0], mean[1])
    std_sel = jnp.where(is0, std[0], std[1])
    dm = (env - mean_sel) / std_sel

    dm = dm.reshape(nf * nloc, nnei, 4)
    M = w2.shape[-1]
    xyz = jnp.zeros((nf * nloc, 4, M), dm.dtype)
    for t in range(NTYPES):
        rr = dm[:, SEC[t] : SEC[t + 1], :]
        x = rr[:, :, :1]
        for W, b in ((w0[t], b0[t]), (w1[t], b1[t]), (w2[t], b2[t])):
            y = jnp.tanh(x @ W + b)
            if W.shape[-1] == 2 * x.shape[-1]:
                y = y + jnp.concatenate([x, x], axis=-1)
            elif W.shape[-1] == x.shape[-1]:
                y = y + x
            x = y
        xyz = xyz + jnp.einsum("nsc,nsm->ncm", rr, x)
    xyz = xyz / NNEI
    xyz = xyz.reshape(nf, nloc, 4, M)
    # int8 with per-atom absmax scale: halves the (latency-bound) fetch
    scale = jnp.max(jnp.abs(xyz), axis=(2, 3))  # [nf, nloc]
    safe = jnp.where(scale > 0, scale, 1.0)
    q = jnp.clip(jnp.round(xyz * (127.0 / safe)[:, :, None, None]),
                 -127, 127).astype(jnp.int8)
    return q, safe.astype(jnp.float32)


_pmapped = None


def _get_pmapped():
    global _pmapped
    if _pmapped is None:
        _pmapped = jax.pmap(_shard_fn, devices=jax.devices()[:NCORES])
    return _pmapped


def kernel(nlist, extended_coord, extended_atype, mean, stddev,
           w0, b0, w1, b1, w2, b2):
    nlist = np.asarray(nlist)
    coord = np.asarray(extended_coord, dtype=np.float32)  # [nf, nall, 3]
    atype = np.asarray(extended_atype)

    # frame offset baked into the indices; padding stays negative (int16 ok:
    # max index 2*8192-1 = 16383 < 32767)
    frame_off = (np.arange(NF, dtype=np.int64) * NALL)[:, None, None]
    nl_abs = np.where(nlist >= 0, nlist + frame_off, -1).astype(np.int16)

    nl_sh = np.ascontiguousarray(
        nl_abs.reshape(NF, NCORES, SHARD, NNEI).transpose(1, 0, 2, 3))
    centers_sh = np.ascontiguousarray(
        coord[:, :NLOC].reshape(NF, NCORES, SHARD, 3).transpose(1, 0, 2, 3))
    atype_sh = np.ascontiguousarray(
        atype[:, :NLOC].astype(np.int32).reshape(NF, NCORES, SHARD)
        .transpose(1, 0, 2))

    coord_all = coord.reshape(NF * NALL, 3)

    def rep(x):
        x = np.asarray(x, dtype=np.float32)
        return np.broadcast_to(x, (NCORES,) + x.shape)

    q, scale = _get_pmapped()(
        nl_sh, rep(coord_all), centers_sh, atype_sh,
        rep(mean), rep(stddev),
        rep(w0), rep(b0), rep(w1), rep(b1), rep(w2), rep(b2),
    )  # q [8, nf, shard, 4, M] int8, scale [8, nf, shard] f32

    q = np.asarray(q)
    scale = np.asarray(scale)  # [8, 2, 512]
    M = q.shape[-1]
    xyz = q.astype(np.float32) * (scale / 127.0)[:, :, :, None, None]
    xyz = xyz.transpose(1, 0, 2, 3, 4).reshape(NF * NLOC, 4, M)
    # res[n, m, a] = sum_c xyz[n, c, m] * xyz[n, c, a] for a < AXIS
    res = np.matmul(xyz.transpose(0, 2, 1), xyz[:, :, :AXIS])
    return np.ascontiguousarray(
        res.reshape(NF, NLOC, M * AXIS).astype(np.float32))


# revision 11
# speedup vs baseline: 1.0875x; 1.0875x over previous
"""DescrptSeA descriptor kernel for 8 Trainium2 NeuronCores.

Data-parallel over the nloc axis (512 atoms/core). The neighbor gather runs
on-device via a flat jnp.take (the take_along_axis form trips a neuron
compiler assert; the flat form compiles). Wire traffic is minimized: in go
int16 neighbor indices with the mask folded into the sign bit (~2.3 MB),
replicated coords (~1.6 MB) and tiny weight tables; out comes only the
rank-4 factor xyz = rr^T @ gg per atom ([nf, 512, 4, 100] bf16, ~6.5 MB)
instead of the full 26-52 MB descriptor. The final res = xyz^T @ xyz[:, :16]
outer product is cheap (52 MFLOP) and runs on host BLAS in fp32.
"""

import numpy as np
import jax
import jax.numpy as jnp

NF, NLOC, NALL = 2, 4096, 8192
NTYPES = 2
SEL = [46, 92]
NNEI = sum(SEL)
SEC = [0, 46, 138]
NEURON = [25, 50, 100]
AXIS = 16
RCUT, RCUT_SMTH = 6.0, 0.5
PROT = 1e-6

NCORES = 8
SHARD = NLOC // NCORES  # 512 atoms per core


def _smooth_weight(d, rmin, rmax):
    uu = (d - rmin) / (rmax - rmin)
    uu = jnp.clip(uu, 0.0, 1.0)
    return uu * uu * uu * (-6.0 * uu * uu + 15.0 * uu - 10.0) + 1.0


def _shard_fn(nl_i16, coord_all, centers, atype_loc, mean, std,
              w0, b0, w1, b1, w2, b2):
    # nl_i16 [nf, shard, nnei] int16: frame-offset index, negative = padded
    # coord_all [nf*nall, 3] f32; centers [nf, shard, 3]
    nf, nloc, nnei = nl_i16.shape
    mask = (nl_i16 >= 0)
    nl = jnp.where(mask, nl_i16, 0).astype(jnp.int32)
    m = mask[..., None].astype(jnp.float32)

    coord_r = jnp.take(coord_all, nl.reshape(-1), axis=0)
    coord_r = coord_r.reshape(nf, nloc, nnei, 3)
    diff = coord_r - centers[:, :, None, :]
    length = jnp.sqrt(jnp.sum(diff * diff, axis=-1, keepdims=True))
    length = length * m + (1.0 - m)
    t0 = 1.0 / (length + PROT)
    t1 = diff / ((length + PROT) ** 2)
    w = _smooth_weight(length, RCUT_SMTH, RCUT) * m
    env = jnp.concatenate([t0, t1], axis=-1) * w  # [nf, shard, nnei, 4]

    is0 = (atype_loc == 0)[:, :, None, None]
    mean_sel = jnp.where(is0, mean[0], mean[1])
    std_sel = jnp.where(is0, std[0], std[1])
    dm = (env - mean_sel) / std_sel

    dm = dm.reshape(nf * nloc, nnei, 4)
    M = w2.shape[-1]
    xyz = jnp.zeros((nf * nloc, 4, M), dm.dtype)
    for t in range(NTYPES):
        rr = dm[:, SEC[t] : SEC[t + 1], :]
        x = rr[:, :, :1]
        for W, b in ((w0[t], b0[t]), (w1[t], b1[t]), (w2[t], b2[t])):
            y = jnp.tanh(x @ W + b)
            if W.shape[-1] == 2 * x.shape[-1]:
                y = y + jnp.concatenate([x, x], axis=-1)
            elif W.shape[-1] == x.shape[-1]:
                y = y + x
            x = y
        xyz = xyz + jnp.einsum("nsc,nsm->ncm", rr, x)
    xyz = xyz / NNEI
    return xyz.reshape(nf, nloc, 4, M).astype(jnp.bfloat16)


_pmapped = None


def _get_pmapped():
    global _pmapped
    if _pmapped is None:
        _pmapped = jax.pmap(_shard_fn, devices=jax.devices()[:NCORES])
    return _pmapped


def kernel(nlist, extended_coord, extended_atype, mean, stddev,
           w0, b0, w1, b1, w2, b2):
    nlist = np.asarray(nlist)
    coord = np.asarray(extended_coord, dtype=np.float32)  # [nf, nall, 3]
    atype = np.asarray(extended_atype)

    # frame offset baked into the indices; padding stays negative (int16 ok:
    # max index 2*8192-1 = 16383 < 32767)
    frame_off = (np.arange(NF, dtype=np.int64) * NALL)[:, None, None]
    nl_abs = np.where(nlist >= 0, nlist + frame_off, -1).astype(np.int16)

    nl_sh = np.ascontiguousarray(
        nl_abs.reshape(NF, NCORES, SHARD, NNEI).transpose(1, 0, 2, 3))
    centers_sh = np.ascontiguousarray(
        coord[:, :NLOC].reshape(NF, NCORES, SHARD, 3).transpose(1, 0, 2, 3))
    atype_sh = np.ascontiguousarray(
        atype[:, :NLOC].astype(np.int32).reshape(NF, NCORES, SHARD)
        .transpose(1, 0, 2))

    coord_all = coord.reshape(NF * NALL, 3)

    def rep(x):
        x = np.asarray(x, dtype=np.float32)
        return np.broadcast_to(x, (NCORES,) + x.shape)

    xyz = _get_pmapped()(
        nl_sh, rep(coord_all), centers_sh, atype_sh,
        rep(mean), rep(stddev),
        rep(w0), rep(b0), rep(w1), rep(b1), rep(w2), rep(b2),
    )  # [8, nf, shard, 4, M] bf16

    xyz = np.asarray(xyz).astype(np.float32)  # [8, 2, 512, 4, 100]
    M = xyz.shape[-1]
    xyz = xyz.transpose(1, 0, 2, 3, 4).reshape(NF * NLOC, 4, M)
    # res[n, m, a] = sum_c xyz[n, c, m] * xyz[n, c, a] for a < AXIS
    res = np.matmul(xyz.transpose(0, 2, 1), xyz[:, :, :AXIS])
    return np.ascontiguousarray(
        res.reshape(NF, NLOC, M * AXIS).astype(np.float32))


# revision 13
# speedup vs baseline: 1.2289x; 1.1300x over previous
"""DescrptSeA descriptor kernel for 8 Trainium2 NeuronCores.

Data-parallel over the nloc axis (512 atoms/core). The neighbor gather runs
on-device via a flat jnp.take (the take_along_axis form trips a neuron
compiler assert; the flat form compiles). Wire traffic is minimized: in go
int16 neighbor indices with the mask folded into the sign bit (~2.3 MB),
replicated coords (~1.6 MB) and tiny weight tables; out comes only the
rank-4 factor xyz = rr^T @ gg per atom ([nf, 512, 4, 100] bf16, ~6.5 MB)
instead of the full 26-52 MB descriptor. The final res = xyz^T @ xyz[:, :16]
outer product is cheap (52 MFLOP) and runs on host BLAS in fp32.
"""

import numpy as np
import jax
import jax.numpy as jnp

NF, NLOC, NALL = 2, 4096, 8192
NTYPES = 2
SEL = [46, 92]
NNEI = sum(SEL)
SEC = [0, 46, 138]
NEURON = [25, 50, 100]
AXIS = 16
RCUT, RCUT_SMTH = 6.0, 0.5
PROT = 1e-6

NCORES = 8
SHARD = NLOC // NCORES  # 512 atoms per core


def _smooth_weight(d, rmin, rmax):
    uu = (d - rmin) / (rmax - rmin)
    uu = jnp.clip(uu, 0.0, 1.0)
    return uu * uu * uu * (-6.0 * uu * uu + 15.0 * uu - 10.0) + 1.0


def _shard_fn(nl_i16, coord_all, centers, atype_loc, mean, std,
              w0, b0, w1, b1, w2, b2):
    # nl_i16 [nf, shard, nnei] int16: frame-offset index, negative = padded
    # coord_all [nf*nall, 3] f32; centers [nf, shard, 3]
    nf, nloc, nnei = nl_i16.shape
    mask = (nl_i16 >= 0)
    nl = jnp.where(mask, nl_i16, 0).astype(jnp.int32)
    m = mask[..., None].astype(jnp.float32)

    coord_r = jnp.take(coord_all, nl.reshape(-1), axis=0)
    coord_r = coord_r.reshape(nf, nloc, nnei, 3)
    diff = coord_r - centers[:, :, None, :]
    length = jnp.sqrt(jnp.sum(diff * diff, axis=-1, keepdims=True))
    length = length * m + (1.0 - m)
    t0 = 1.0 / (length + PROT)
    t1 = diff / ((length + PROT) ** 2)
    w = _smooth_weight(length, RCUT_SMTH, RCUT) * m
    env = jnp.concatenate([t0, t1], axis=-1) * w  # [nf, shard, nnei, 4]

    is0 = (atype_loc == 0)[:, :, None, None]
    mean_sel = jnp.where(is0, mean[0], mean[1])
    std_sel = jnp.where(is0, std[0], std[1])
    dm = (env - mean_sel) / std_sel

    dm = dm.reshape(nf * nloc, nnei, 4)
    M = w2.shape[-1]
    xyz = jnp.zeros((nf * nloc, 4, M), dm.dtype)
    for t in range(NTYPES):
        rr = dm[:, SEC[t] : SEC[t + 1], :]
        x = rr[:, :, :1]
        for W, b in ((w0[t], b0[t]), (w1[t], b1[t]), (w2[t], b2[t])):
            y = jnp.tanh(x @ W + b)
            if W.shape[-1] == 2 * x.shape[-1]:
                y = y + jnp.concatenate([x, x], axis=-1)
            elif W.shape[-1] == x.shape[-1]:
                y = y + x
            x = y
        xyz = xyz + jnp.einsum("nsc,nsm->ncm", rr, x)
    xyz = xyz / NNEI
    xyz = xyz.reshape(nf, nloc, 4, M)
    # Single-buffer int8 packing: the tunnel fetch has a ~4MB chunk cliff,
    # so 3.2MB int8 beats 6.6MB bf16. Per-atom scale is carried in the same
    # buffer as (mantissa, exponent) int8 pair built arithmetically (a
    # bitcast+concat encoding trips a neuron-compiler assert). Quantizing
    # against the *quantized* scale keeps the error identical to exact-scale
    # int8; ceil on the mantissa guarantees scale_q >= absmax (no clipping).
    amax = jnp.max(jnp.abs(xyz), axis=(2, 3))  # [nf, nloc]
    safe = jnp.where(amax > 0, amax, 1.0)
    e = jnp.floor(jnp.log2(safe))
    m = safe * jnp.exp2(-e)  # [1, 2)
    mq = jnp.ceil((m - 1.0) * 127.0)  # [0, 127]
    scale_q = (1.0 + mq / 127.0) * jnp.exp2(e)
    q = jnp.clip(jnp.round(xyz * (127.0 / scale_q)[:, :, None, None]),
                 -127, 127).astype(jnp.int8).reshape(nf, nloc, 4 * M)
    return jnp.concatenate(
        [q, mq.astype(jnp.int8)[..., None], e.astype(jnp.int8)[..., None]],
        axis=-1)  # [nf, nloc, 4*M+2] int8


_pmapped = None


def _get_pmapped():
    global _pmapped
    if _pmapped is None:
        _pmapped = jax.pmap(_shard_fn, devices=jax.devices()[:NCORES])
    return _pmapped


def kernel(nlist, extended_coord, extended_atype, mean, stddev,
           w0, b0, w1, b1, w2, b2):
    nlist = np.asarray(nlist)
    coord = np.asarray(extended_coord, dtype=np.float32)  # [nf, nall, 3]
    atype = np.asarray(extended_atype)

    # frame offset baked into the indices; padding stays negative (int16 ok:
    # max index 2*8192-1 = 16383 < 32767)
    frame_off = (np.arange(NF, dtype=np.int64) * NALL)[:, None, None]
    nl_abs = np.where(nlist >= 0, nlist + frame_off, -1).astype(np.int16)

    nl_sh = np.ascontiguousarray(
        nl_abs.reshape(NF, NCORES, SHARD, NNEI).transpose(1, 0, 2, 3))
    centers_sh = np.ascontiguousarray(
        coord[:, :NLOC].reshape(NF, NCORES, SHARD, 3).transpose(1, 0, 2, 3))
    atype_sh = np.ascontiguousarray(
        atype[:, :NLOC].astype(np.int32).reshape(NF, NCORES, SHARD)
        .transpose(1, 0, 2))

    coord_all = coord.reshape(NF * NALL, 3)

    def rep(x):
        x = np.asarray(x, dtype=np.float32)
        return np.broadcast_to(x, (NCORES,) + x.shape)

    packed = _get_pmapped()(
        nl_sh, rep(coord_all), centers_sh, atype_sh,
        rep(mean), rep(stddev),
        rep(w0), rep(b0), rep(w1), rep(b1), rep(w2), rep(b2),
    )  # [8, nf, shard, 4*M+2] int8

    packed = np.asarray(packed)
    M = NEURON[-1]
    mq = packed[..., 4 * M].astype(np.float32)
    e = packed[..., 4 * M + 1].astype(np.float32)
    scale_q = (1.0 + mq / 127.0) * np.exp2(e)  # [8, 2, 512]
    xyz = packed[..., : 4 * M].astype(np.float32).reshape(
        NCORES, NF, SHARD, 4, M) * (scale_q / 127.0)[:, :, :, None, None]
    xyz = xyz.transpose(1, 0, 2, 3, 4).reshape(NF * NLOC, 4, M)
    # res[n, m, a] = sum_c xyz[n, c, m] * xyz[n, c, a] for a < AXIS
    res = np.matmul(xyz.transpose(0, 2, 1), xyz[:, :, :AXIS])
    return np.ascontiguousarray(
        res.reshape(NF, NLOC, M * AXIS).astype(np.float32))


# revision 16
# speedup vs baseline: 1.3873x; 1.1289x over previous
"""DescrptSeA descriptor kernel for 8 Trainium2 NeuronCores.

Data-parallel over the nloc axis (512 atoms/core). The neighbor gather runs
on-device via a flat jnp.take (the take_along_axis form trips a neuron
compiler assert; the flat form compiles). Wire traffic is minimized: in go
int16 neighbor indices with the mask folded into the sign bit (~2.3 MB),
replicated coords (~1.6 MB) and tiny weight tables; out comes only the
rank-4 factor xyz = rr^T @ gg per atom ([nf, 512, 4, 100] bf16, ~6.5 MB)
instead of the full 26-52 MB descriptor. The final res = xyz^T @ xyz[:, :16]
outer product is cheap (52 MFLOP) and runs on host BLAS in fp32.
"""

import numpy as np
import jax
import jax.numpy as jnp

NF, NLOC, NALL = 2, 4096, 8192
NTYPES = 2
SEL = [46, 92]
NNEI = sum(SEL)
SEC = [0, 46, 138]
NEURON = [25, 50, 100]
AXIS = 16
RCUT, RCUT_SMTH = 6.0, 0.5
PROT = 1e-6

NCORES = 8
SHARD = NLOC // NCORES  # 512 atoms per core


def _smooth_weight(d, rmin, rmax):
    uu = (d - rmin) / (rmax - rmin)
    uu = jnp.clip(uu, 0.0, 1.0)
    return uu * uu * uu * (-6.0 * uu * uu + 15.0 * uu - 10.0) + 1.0


def _shard_fn(nl_i16, coord_q, centers_q, cparams, atype_loc, mean, std,
              w0, b0, w1, b1, w2, b2):
    # nl_i16 [nf, shard, nnei] int16: frame-offset index, negative = padded
    # coord_q [nf*nall, 3] int16 fixed-point; centers_q [nf, shard, 3] int16
    # cparams [2] f32: (cmin, step) for exact affine dequantization
    nf, nloc, nnei = nl_i16.shape
    mask = (nl_i16 >= 0)
    nl = jnp.where(mask, nl_i16, 0).astype(jnp.int32)
    m = mask[..., None].astype(jnp.float32)

    step = cparams[1]
    coord_all = (coord_q.astype(jnp.float32) + 32768.0) * step
    centers = (centers_q.astype(jnp.float32) + 32768.0) * step
    coord_r = jnp.take(coord_all, nl.reshape(-1), axis=0)
    coord_r = coord_r.reshape(nf, nloc, nnei, 3)
    diff = coord_r - centers[:, :, None, :]  # cmin offset cancels here
    length = jnp.sqrt(jnp.sum(diff * diff, axis=-1, keepdims=True))
    length = length * m + (1.0 - m)
    t0 = 1.0 / (length + PROT)
    t1 = diff / ((length + PROT) ** 2)
    w = _smooth_weight(length, RCUT_SMTH, RCUT) * m
    env = jnp.concatenate([t0, t1], axis=-1) * w  # [nf, shard, nnei, 4]

    is0 = (atype_loc == 0)[:, :, None, None]
    mean_sel = jnp.where(is0, mean[0], mean[1])
    std_sel = jnp.where(is0, std[0], std[1])
    dm = (env - mean_sel) / std_sel

    dm = dm.reshape(nf * nloc, nnei, 4)
    M = w2.shape[-1]
    xyz = jnp.zeros((nf * nloc, 4, M), dm.dtype)
    for t in range(NTYPES):
        rr = dm[:, SEC[t] : SEC[t + 1], :]
        x = rr[:, :, :1]
        for W, b in ((w0[t], b0[t]), (w1[t], b1[t]), (w2[t], b2[t])):
            y = jnp.tanh(x @ W + b)
            if W.shape[-1] == 2 * x.shape[-1]:
                y = y + jnp.concatenate([x, x], axis=-1)
            elif W.shape[-1] == x.shape[-1]:
                y = y + x
            x = y
        xyz = xyz + jnp.einsum("nsc,nsm->ncm", rr, x)
    xyz = xyz / NNEI
    xyz = xyz.reshape(nf, nloc, 4, M)
    # Single-buffer int8 packing: the tunnel fetch has a ~4MB chunk cliff,
    # so 3.2MB int8 beats 6.6MB bf16. Per-atom scale is carried in the same
    # buffer as (mantissa, exponent) int8 pair built arithmetically (a
    # bitcast+concat encoding trips a neuron-compiler assert). Quantizing
    # against the *quantized* scale keeps the error identical to exact-scale
    # int8; ceil on the mantissa guarantees scale_q >= absmax (no clipping).
    amax = jnp.max(jnp.abs(xyz), axis=(2, 3))  # [nf, nloc]
    safe = jnp.where(amax > 0, amax, 1.0)
    e = jnp.floor(jnp.log2(safe))
    m = safe * jnp.exp2(-e)  # [1, 2)
    mq = jnp.ceil((m - 1.0) * 127.0)  # [0, 127]
    scale_q = (1.0 + mq / 127.0) * jnp.exp2(e)
    q = jnp.clip(jnp.round(xyz * (127.0 / scale_q)[:, :, None, None]),
                 -127, 127).astype(jnp.int8).reshape(nf, nloc, 4 * M)
    return jnp.concatenate(
        [q, mq.astype(jnp.int8)[..., None], e.astype(jnp.int8)[..., None]],
        axis=-1)  # [nf, nloc, 4*M+2] int8


_pmapped = None


def _get_pmapped():
    global _pmapped
    if _pmapped is None:
        _pmapped = jax.pmap(_shard_fn, devices=jax.devices()[:NCORES])
    return _pmapped


def kernel(nlist, extended_coord, extended_atype, mean, stddev,
           w0, b0, w1, b1, w2, b2):
    nlist = np.asarray(nlist)
    coord = np.asarray(extended_coord, dtype=np.float32)  # [nf, nall, 3]
    atype = np.asarray(extended_atype)

    # frame offset baked into the indices; padding stays negative (int16 ok:
    # max index 2*8192-1 = 16383 < 32767)
    frame_off = (np.arange(NF, dtype=np.int64) * NALL)[:, None, None]
    nl_abs = np.where(nlist >= 0, nlist + frame_off, -1).astype(np.int16)

    nl_sh = np.ascontiguousarray(
        nl_abs.reshape(NF, NCORES, SHARD, NNEI).transpose(1, 0, 2, 3))
    atype_sh = np.ascontiguousarray(
        atype[:, :NLOC].astype(np.int32).reshape(NF, NCORES, SHARD)
        .transpose(1, 0, 2))

    # int16 fixed-point coords: halves the push; exact affine dequant on
    # device, and the offset cancels in the neighbor-center subtraction
    cmin = float(coord.min())
    span = max(float(coord.max()) - cmin, 1e-9)
    step = span / 65535.0
    qs = 1.0 / step
    coord_q = np.clip(np.round((coord - cmin) * qs) - 32768.0,
                      -32768, 32767).astype(np.int16)  # [nf, nall, 3]
    coord_all_q = coord_q.reshape(NF * NALL, 3)
    centers_q = np.ascontiguousarray(
        coord_q[:, :NLOC].reshape(NF, NCORES, SHARD, 3).transpose(1, 0, 2, 3))
    cparams = np.array([cmin, step], dtype=np.float32)

    def rep(x):
        x = np.asarray(x, dtype=np.float32)
        return np.broadcast_to(x, (NCORES,) + x.shape)

    packed = _get_pmapped()(
        nl_sh,
        np.broadcast_to(coord_all_q, (NCORES,) + coord_all_q.shape),
        centers_q, rep(cparams), atype_sh,
        rep(mean), rep(stddev),
        rep(w0), rep(b0), rep(w1), rep(b1), rep(w2), rep(b2),
    )  # [8, nf, shard, 4*M+2] int8

    packed = np.asarray(packed)
    M = NEURON[-1]
    mq = packed[..., 4 * M].astype(np.float32)
    e = packed[..., 4 * M + 1].astype(np.float32)
    scale_q = (1.0 + mq / 127.0) * np.exp2(e)  # [8, 2, 512]
    xyz = packed[..., : 4 * M].astype(np.float32).reshape(
        NCORES, NF, SHARD, 4, M) * (scale_q / 127.0)[:, :, :, None, None]
    xyz = xyz.transpose(1, 0, 2, 3, 4).reshape(NF * NLOC, 4, M)
    # res[n, m, a] = sum_c xyz[n, c, m] * xyz[n, c, a] for a < AXIS
    res = np.matmul(xyz.transpose(0, 2, 1), xyz[:, :, :AXIS])
    return np.ascontiguousarray(
        res.reshape(NF, NLOC, M * AXIS).astype(np.float32))


# revision 17
# speedup vs baseline: 1.4770x; 1.0646x over previous
"""DescrptSeA descriptor kernel for 8 Trainium2 NeuronCores.

Data-parallel over the nloc axis (512 atoms/core). The neighbor gather runs
on-device via a flat jnp.take (the take_along_axis form trips a neuron
compiler assert; the flat form compiles). Wire traffic is minimized: in go
int16 neighbor indices with the mask folded into the sign bit (~2.3 MB),
replicated coords (~1.6 MB) and tiny weight tables; out comes only the
rank-4 factor xyz = rr^T @ gg per atom ([nf, 512, 4, 100] bf16, ~6.5 MB)
instead of the full 26-52 MB descriptor. The final res = xyz^T @ xyz[:, :16]
outer product is cheap (52 MFLOP) and runs on host BLAS in fp32.
"""

import numpy as np
import jax
import jax.numpy as jnp

NF, NLOC, NALL = 2, 4096, 8192
NTYPES = 2
SEL = [46, 92]
NNEI = sum(SEL)
SEC = [0, 46, 138]
NEURON = [25, 50, 100]
AXIS = 16
RCUT, RCUT_SMTH = 6.0, 0.5
PROT = 1e-6

NCORES = 8
SHARD = NLOC // NCORES  # 512 atoms per core


def _smooth_weight(d, rmin, rmax):
    uu = (d - rmin) / (rmax - rmin)
    uu = jnp.clip(uu, 0.0, 1.0)
    return uu * uu * uu * (-6.0 * uu * uu + 15.0 * uu - 10.0) + 1.0


def _shard_fn(nl_i16, coord_q, centers_q, cparams, atype_loc, mean, std,
              w0, b0, w1, b1, w2, b2):
    # nl_i16 [nf, shard, nnei] int16: frame-offset index, negative = padded
    # coord_q [nf*nall, 3] int16 fixed-point; centers_q [nf, shard, 3] int16
    # cparams [2] f32: (cmin, step) for exact affine dequantization
    nf, nloc, nnei = nl_i16.shape
    mask = (nl_i16 >= 0)
    nl = jnp.where(mask, nl_i16, 0).astype(jnp.int32)
    m = mask[..., None].astype(jnp.float32)

    step = cparams[1]
    coord_all = (coord_q.astype(jnp.float32) + 32768.0) * step
    centers = (centers_q.astype(jnp.float32) + 32768.0) * step
    coord_r = jnp.take(coord_all, nl.reshape(-1), axis=0)
    coord_r = coord_r.reshape(nf, nloc, nnei, 3)
    diff = coord_r - centers[:, :, None, :]  # cmin offset cancels here
    length = jnp.sqrt(jnp.sum(diff * diff, axis=-1, keepdims=True))
    length = length * m + (1.0 - m)
    t0 = 1.0 / (length + PROT)
    t1 = diff / ((length + PROT) ** 2)
    w = _smooth_weight(length, RCUT_SMTH, RCUT) * m
    env = jnp.concatenate([t0, t1], axis=-1) * w  # [nf, shard, nnei, 4]

    is0 = (atype_loc == 0)[:, :, None, None]
    mean_sel = jnp.where(is0, mean[0], mean[1])
    std_sel = jnp.where(is0, std[0], std[1])
    dm = (env - mean_sel) / std_sel

    dm = dm.reshape(nf * nloc, nnei, 4)
    M = w2.shape[-1]
    xyz = jnp.zeros((nf * nloc, 4, M), dm.dtype)
    for t in range(NTYPES):
        rr = dm[:, SEC[t] : SEC[t + 1], :]
        x = rr[:, :, :1]
        for W, b in ((w0[t], b0[t]), (w1[t], b1[t]), (w2[t], b2[t])):
            y = jnp.tanh(x @ W + b)
            if W.shape[-1] == 2 * x.shape[-1]:
                y = y + jnp.concatenate([x, x], axis=-1)
            elif W.shape[-1] == x.shape[-1]:
                y = y + x
            x = y
        xyz = xyz + jnp.einsum("nsc,nsm->ncm", rr, x)
    xyz = xyz / NNEI
    xyz = xyz.reshape(nf, nloc, 4, M)
    # Single-buffer int8 packing: the tunnel fetch has a ~4MB chunk cliff,
    # so 3.2MB int8 beats 6.6MB bf16. Per-atom scale is carried in the same
    # buffer as (mantissa, exponent) int8 pair built arithmetically (a
    # bitcast+concat encoding trips a neuron-compiler assert). Quantizing
    # against the *quantized* scale keeps the error identical to exact-scale
    # int8; ceil on the mantissa guarantees scale_q >= absmax (no clipping).
    amax = jnp.max(jnp.abs(xyz), axis=(2, 3))  # [nf, nloc]
    safe = jnp.where(amax > 0, amax, 1.0)
    e = jnp.floor(jnp.log2(safe))
    m = safe * jnp.exp2(-e)  # [1, 2)
    mq = jnp.ceil((m - 1.0) * 127.0)  # [0, 127]
    scale_q = (1.0 + mq / 127.0) * jnp.exp2(e)
    q = jnp.clip(jnp.round(xyz * (127.0 / scale_q)[:, :, None, None]),
                 -127, 127).astype(jnp.int8).reshape(nf, nloc, 4 * M)
    return jnp.concatenate(
        [q, mq.astype(jnp.int8)[..., None], e.astype(jnp.int8)[..., None]],
        axis=-1)  # [nf, nloc, 4*M+2] int8


_pmapped = None


def _get_pmapped():
    global _pmapped
    if _pmapped is None:
        _pmapped = jax.pmap(_shard_fn, devices=jax.devices()[:NCORES])
    return _pmapped


def kernel(nlist, extended_coord, extended_atype, mean, stddev,
           w0, b0, w1, b1, w2, b2):
    nlist = np.asarray(nlist)
    coord = np.asarray(extended_coord, dtype=np.float32)  # [nf, nall, 3]
    atype = np.asarray(extended_atype)

    # frame offset baked into the indices; padding stays negative (int16 ok:
    # max index 2*8192-1 = 16383 < 32767)
    frame_off = (np.arange(NF, dtype=np.int64) * NALL)[:, None, None]
    nl_abs = np.where(nlist >= 0, nlist + frame_off, -1).astype(np.int16)

    nl_sh = np.ascontiguousarray(
        nl_abs.reshape(NF, NCORES, SHARD, NNEI).transpose(1, 0, 2, 3))
    atype_sh = np.ascontiguousarray(
        atype[:, :NLOC].astype(np.int32).reshape(NF, NCORES, SHARD)
        .transpose(1, 0, 2))

    # int16 fixed-point coords: halves the push; exact affine dequant on
    # device, and the offset cancels in the neighbor-center subtraction
    cmin = float(coord.min())
    span = max(float(coord.max()) - cmin, 1e-9)
    step = span / 65535.0
    qs = 1.0 / step
    coord_q = np.clip(np.round((coord - cmin) * qs) - 32768.0,
                      -32768, 32767).astype(np.int16)  # [nf, nall, 3]
    coord_all_q = coord_q.reshape(NF * NALL, 3)
    centers_q = np.ascontiguousarray(
        coord_q[:, :NLOC].reshape(NF, NCORES, SHARD, 3).transpose(1, 0, 2, 3))
    cparams = np.array([cmin, step], dtype=np.float32)

    def rep(x):
        x = np.asarray(x, dtype=np.float32)
        return np.broadcast_to(x, (NCORES,) + x.shape)

    packed = _get_pmapped()(
        nl_sh,
        np.broadcast_to(coord_all_q, (NCORES,) + coord_all_q.shape),
        centers_q, rep(cparams), atype_sh,
        rep(mean), rep(stddev),
        rep(w0), rep(b0), rep(w1), rep(b1), rep(w2), rep(b2),
    )  # [8, nf, shard, 4*M+2] int8

    packed = np.asarray(packed)
    M = NEURON[-1]
    mq = packed[..., 4 * M].astype(np.float32)
    e = packed[..., 4 * M + 1].astype(np.float32)
    scale_q = (1.0 + mq / 127.0) * np.exp2(e)  # [8, 2, 512]
    xyz = packed[..., : 4 * M].astype(np.float32).reshape(
        NCORES, NF, SHARD, 4, M) * (scale_q / 127.0)[:, :, :, None, None]
    xyz = xyz.transpose(1, 0, 2, 3, 4).reshape(NF * NLOC, 4, M)
    # res[n, m, a] = sum_c xyz[n, c, m] * xyz[n, c, a] for a < AXIS
    res = np.matmul(xyz.transpose(0, 2, 1), xyz[:, :, :AXIS])
    # res is already contiguous f32 from np.matmul: avoid a 52 MB copy
    return res.reshape(NF, NLOC, M * AXIS)


# revision 18
# speedup vs baseline: 1.5122x; 1.0238x over previous
"""DescrptSeA descriptor kernel for 8 Trainium2 NeuronCores.

Data-parallel over the nloc axis (512 atoms/core). The neighbor gather runs
on-device via a flat jnp.take (the take_along_axis form trips a neuron
compiler assert; the flat form compiles). Wire traffic is minimized: in go
int16 neighbor indices with the mask folded into the sign bit (~2.3 MB),
replicated coords (~1.6 MB) and tiny weight tables; out comes only the
rank-4 factor xyz = rr^T @ gg per atom ([nf, 512, 4, 100] bf16, ~6.5 MB)
instead of the full 26-52 MB descriptor. The final res = xyz^T @ xyz[:, :16]
outer product is cheap (52 MFLOP) and runs on host BLAS in fp32.
"""

import numpy as np
import jax
import jax.numpy as jnp

NF, NLOC, NALL = 2, 4096, 8192
NTYPES = 2
SEL = [46, 92]
NNEI = sum(SEL)
SEC = [0, 46, 138]
NEURON = [25, 50, 100]
AXIS = 16
RCUT, RCUT_SMTH = 6.0, 0.5
PROT = 1e-6

NCORES = 8
SHARD = NLOC // NCORES  # 512 atoms per core


def _smooth_weight(d, rmin, rmax):
    uu = (d - rmin) / (rmax - rmin)
    uu = jnp.clip(uu, 0.0, 1.0)
    return uu * uu * uu * (-6.0 * uu * uu + 15.0 * uu - 10.0) + 1.0


def _shard_fn(nl_i16, coord_q, centers_q, cparams, atype_loc, mean, std,
              w0, b0, w1, b1, w2, b2):
    # nl_i16 [nf, shard, nnei] int16: frame-offset index, negative = padded
    # coord_q [nf*nall, 3] int16 fixed-point; centers_q [nf, shard, 3] int16
    # cparams [2] f32: (cmin, step) for exact affine dequantization
    nf, nloc, nnei = nl_i16.shape
    mask = (nl_i16 >= 0)
    nl = jnp.where(mask, nl_i16, 0).astype(jnp.int32)
    m = mask[..., None].astype(jnp.float32)

    step = cparams[1]
    coord_all = (coord_q.astype(jnp.float32) + 32768.0) * step
    centers = (centers_q.astype(jnp.float32) + 32768.0) * step
    coord_r = jnp.take(coord_all, nl.reshape(-1), axis=0)
    coord_r = coord_r.reshape(nf, nloc, nnei, 3)
    diff = coord_r - centers[:, :, None, :]  # cmin offset cancels here
    length = jnp.sqrt(jnp.sum(diff * diff, axis=-1, keepdims=True))
    length = length * m + (1.0 - m)
    t0 = 1.0 / (length + PROT)
    t1 = diff / ((length + PROT) ** 2)
    w = _smooth_weight(length, RCUT_SMTH, RCUT) * m
    env = jnp.concatenate([t0, t1], axis=-1) * w  # [nf, shard, nnei, 4]

    is0 = (atype_loc == 0)[:, :, None, None]
    mean_sel = jnp.where(is0, mean[0], mean[1])
    std_sel = jnp.where(is0, std[0], std[1])
    dm = (env - mean_sel) / std_sel

    dm = dm.reshape(nf * nloc, nnei, 4)
    M = w2.shape[-1]
    xyz = jnp.zeros((nf * nloc, 4, M), dm.dtype)
    for t in range(NTYPES):
        rr = dm[:, SEC[t] : SEC[t + 1], :]
        x = rr[:, :, :1]
        for W, b in ((w0[t], b0[t]), (w1[t], b1[t]), (w2[t], b2[t])):
            y = jnp.tanh(x @ W + b)
            if W.shape[-1] == 2 * x.shape[-1]:
                y = y + jnp.concatenate([x, x], axis=-1)
            elif W.shape[-1] == x.shape[-1]:
                y = y + x
            x = y
        xyz = xyz + jnp.einsum("nsc,nsm->ncm", rr, x)
    xyz = xyz / NNEI
    xyz = xyz.reshape(nf, nloc, 4, M)
    # Single-buffer int8 packing: the tunnel fetch has a ~4MB chunk cliff,
    # so 3.2MB int8 beats 6.6MB bf16. Per-atom scale is carried in the same
    # buffer as (mantissa, exponent) int8 pair built arithmetically (a
    # bitcast+concat encoding trips a neuron-compiler assert). Quantizing
    # against the *quantized* scale keeps the error identical to exact-scale
    # int8; ceil on the mantissa guarantees scale_q >= absmax (no clipping).
    amax = jnp.max(jnp.abs(xyz), axis=(2, 3))  # [nf, nloc]
    safe = jnp.where(amax > 0, amax, 1.0)
    e = jnp.floor(jnp.log2(safe))
    m = safe * jnp.exp2(-e)  # [1, 2)
    mq = jnp.ceil((m - 1.0) * 127.0)  # [0, 127]
    scale_q = (1.0 + mq / 127.0) * jnp.exp2(e)
    q = jnp.clip(jnp.round(xyz * (127.0 / scale_q)[:, :, None, None]),
                 -127, 127).astype(jnp.int8).reshape(nf, nloc, 4 * M)
    return jnp.concatenate(
        [q, mq.astype(jnp.int8)[..., None], e.astype(jnp.int8)[..., None]],
        axis=-1)  # [nf, nloc, 4*M+2] int8


_pmapped = None


def _get_pmapped():
    global _pmapped
    if _pmapped is None:
        _pmapped = jax.pmap(_shard_fn, devices=jax.devices()[:NCORES])
    return _pmapped


def kernel(nlist, extended_coord, extended_atype, mean, stddev,
           w0, b0, w1, b1, w2, b2):
    nlist = np.asarray(nlist)
    coord = np.asarray(extended_coord, dtype=np.float32)  # [nf, nall, 3]
    atype = np.asarray(extended_atype)

    # frame offset baked into the indices; padding stays negative (int16 ok:
    # max index 2*8192-1 = 16383 < 32767)
    frame_off = (np.arange(NF, dtype=np.int64) * NALL)[:, None, None]
    nl_abs = np.where(nlist >= 0, nlist + frame_off, -1).astype(np.int16)

    nl_sh = np.ascontiguousarray(
        nl_abs.reshape(NF, NCORES, SHARD, NNEI).transpose(1, 0, 2, 3))
    atype_sh = np.ascontiguousarray(
        atype[:, :NLOC].astype(np.int32).reshape(NF, NCORES, SHARD)
        .transpose(1, 0, 2))

    # int16 fixed-point coords: halves the push; exact affine dequant on
    # device, and the offset cancels in the neighbor-center subtraction
    cmin = float(coord.min())
    span = max(float(coord.max()) - cmin, 1e-9)
    step = span / 65535.0
    qs = 1.0 / step
    coord_q = np.clip(np.round((coord - cmin) * qs) - 32768.0,
                      -32768, 32767).astype(np.int16)  # [nf, nall, 3]
    coord_all_q = coord_q.reshape(NF * NALL, 3)
    centers_q = np.ascontiguousarray(
        coord_q[:, :NLOC].reshape(NF, NCORES, SHARD, 3).transpose(1, 0, 2, 3))
    cparams = np.array([cmin, step], dtype=np.float32)

    def rep(x):
        x = np.asarray(x, dtype=np.float32)
        return np.broadcast_to(x, (NCORES,) + x.shape)

    packed = _get_pmapped()(
        nl_sh,
        np.broadcast_to(coord_all_q, (NCORES,) + coord_all_q.shape),
        centers_q, rep(cparams), atype_sh,
        rep(mean), rep(stddev),
        rep(w0), rep(b0), rep(w1), rep(b1), rep(w2), rep(b2),
    )  # [8, nf, shard, 4*M+2] int8

    packed = np.asarray(packed)  # [8, nf, shard, 4*M+2] int8
    M = NEURON[-1]
    out = np.empty((NF, NLOC, M * AXIS), np.float32)
    # per-core dequant + res matmul written straight into the output slice:
    # avoids a 13 MB transpose/reshape copy of the interleaved shards
    for c in range(NCORES):
        pc = packed[c]  # [nf, shard, 4*M+2]
        mq = pc[..., 4 * M].astype(np.float32)
        e = pc[..., 4 * M + 1].astype(np.float32)
        sc = (1.0 + mq / 127.0) * np.exp2(e) / 127.0  # [nf, shard]
        xyz = pc[..., : 4 * M].astype(np.float32).reshape(
            NF, SHARD, 4, M) * sc[..., None, None]
        # res[m, a] = sum_ch xyz[ch, m] * xyz[ch, a] for a < AXIS
        res = np.matmul(xyz.transpose(0, 1, 3, 2), xyz[..., :AXIS])
        out[:, c * SHARD : (c + 1) * SHARD] = res.reshape(NF, SHARD, M * AXIS)
    return out
